# revision 4
# baseline (speedup 1.0000x reference)
"""Trainium2 Bass kernel for nn_ActionDetokenizer (gnn_message_passing).

Computes: out[b, j, k] = sum_d x[b, j+1, d] * W[j, d, k] + bias[j, k]
  x: [65536, 13, 256] f32, W: [12, 256, 2] f32, b: [12, 2] f32 -> out [65536, 12, 2] f32

Strategy (pure data parallel over batch, 8 cores):
  - Host: shard batch across 8 cores; per core, relayout the needed slice of x
    to d-major [24=(j,c), 128=d, 8192=b] so the contraction dim (d) lands on
    SBUF partitions (the TensorEngine contracts along partitions).  The tiny
    weight stack is replicated to every core.
  - Device: stream x tiles HBM->SBUF (memory-bound: ~100.7 MB/core), and for
    each (joint, 512-batch chunk) accumulate two K=128 matmuls (d-chunks) into
    PSUM with W[j] chunks as the stationary operand.  ScalarEngine drains
    PSUM->SBUF fusing the per-(j,k) bias add (Identity activation, per
    partition bias AP).  Output is written d..-major [12, 2, 8192] per core and
    re-oriented on the host during the gather step.

Compute paths (KERNEL_PATH env or _PATH):
  f32  : exact fp32 matmuls (4 cyc/row on PE)
  f32r : float32r matmuls (1 cyc/row at N>=256; reduced precision - probe!)
  hilo : x and W split into bf16 hi+lo on host; 3-term bf16 matmuls
         (xhi@whi + xhi@wlo + xlo@whi), ~1e-5 rel err, 1 cyc/row.
"""

import os

import numpy as np

M_CORES = 8
B_FULL = 65536
BL = B_FULL // M_CORES  # 8192 batch rows per core
J = 12  # joints
D = 256  # embed dim
K = 2  # outputs per joint
P = 128  # SBUF partitions / d-chunk
C = D // P  # 2 d-chunks
JC = J * C  # 24 (j, c) planes
NB_TILE = 4096  # batch columns per SBUF x tile
N_MM = 512  # batch columns per matmul (fp32 moving-operand max / PSUM bank)

_PATH = os.environ.get("KERNEL_PATH", "f32")

_CACHE = {}


def _build(path, bl):
    import concourse.bacc as bacc
    import concourse.mybir as mybir
    from concourse.tile import TileContext

    f32 = mybir.dt.float32
    xdt = {
        "f32": f32,
        "f32r": mybir.dt.float32r,
        "hilo": mybir.dt.bfloat16,
    }[path]

    # Bacc (not plain Bass): its compile() legalizes multi-wait instructions
    # into event semaphores / ldweights waits, which walrus codegen requires
    # (at most one wait command per compute instruction on TRN2).
    nc = bacc.Bacc("TRN2", target_bir_lowering=False)

    if path == "hilo":
        x_drams = [
            nc.dram_tensor("xt_hi", [JC, P, bl], xdt, kind="ExternalInput"),
            nc.dram_tensor("xt_lo", [JC, P, bl], xdt, kind="ExternalInput"),
        ]
        w_drams = [
            nc.dram_tensor("wt_hi", [P, JC * K], xdt, kind="ExternalInput"),
            nc.dram_tensor("wt_lo", [P, JC * K], xdt, kind="ExternalInput"),
        ]
    else:
        x_drams = [nc.dram_tensor("xt", [JC, P, bl], xdt, kind="ExternalInput")]
        w_drams = [nc.dram_tensor("wt", [P, JC * K], xdt, kind="ExternalInput")]
    bias_dram = nc.dram_tensor("bias", [K, J], f32, kind="ExternalInput")
    out_dram = nc.dram_tensor("out", [J, K, bl], f32, kind="ExternalOutput")

    nb = min(NB_TILE, bl)
    assert bl % nb == 0 and nb % N_MM == 0
    n_bh = bl // nb
    n_n = nb // N_MM

    with TileContext(nc) as tc:
        with (
            tc.tile_pool(name="wpool", bufs=1) as wpool,
            tc.tile_pool(name="xpool", bufs=6) as xpool,
            tc.tile_pool(name="opool", bufs=3) as opool,
            tc.tile_pool(name="pspool", bufs=8, space="PSUM") as pspool,
        ):
            w_sbs = []
            for wi, wd in enumerate(w_drams):
                w_sb = wpool.tile([P, JC * K], xdt, tag=f"w{wi}")
                nc.sync.dma_start(out=w_sb[:, :], in_=wd[:, :])
                w_sbs.append(w_sb)
            bias_sb = wpool.tile([K, J], f32, tag="bias")
            nc.sync.dma_start(out=bias_sb[:, :], in_=bias_dram[:, :])

            for bh in range(n_bh):
                for j in range(J):
                    # load the two d-chunk tiles for each x source
                    x_tiles = []  # x_tiles[src][c]
                    for si, xd in enumerate(x_drams):
                        per_c = []
                        for c in range(C):
                            t = xpool.tile([P, nb], xdt, tag=f"x{si}{c}")
                            nc.sync.dma_start(
                                out=t[:, :],
                                in_=xd[C * j + c, :, bh * nb : (bh + 1) * nb],
                            )
                            per_c.append(t)
                        x_tiles.append(per_c)

                    ot = opool.tile([K, nb], f32, tag="o")
                    for n in range(n_n):
                        ps = pspool.tile([K, N_MM], f32, tag="ps")
                        # matmul accumulation sequence for this (j, n)
                        if path == "hilo":
                            # (w_src, x_src): hi@hi + lo@hi + hi@lo
                            seq = [(0, 0), (1, 0), (0, 1)]
                        else:
                            seq = [(0, 0)]
                        mms = [(ws, xs, c) for c in range(C) for (ws, xs) in seq]
                        for i, (ws, xs, c) in enumerate(mms):
                            jc = C * j + c
                            nc.tensor.matmul(
                                ps[:, :],
                                lhsT=w_sbs[ws][:, jc * K : (jc + 1) * K],
                                rhs=x_tiles[xs][c][:, n * N_MM : (n + 1) * N_MM],
                                start=(i == 0),
                                stop=(i == len(mms) - 1),
                            )
                        # PSUM -> SBUF with fused per-(j,k) bias add
                        nc.scalar.activation(
                            out=ot[:, n * N_MM : (n + 1) * N_MM],
                            in_=ps[:, :],
                            func=mybir.ActivationFunctionType.Identity,
                            bias=bias_sb[:, j : j + 1],
                            scale=1.0,
                        )
                    nc.sync.dma_start(
                        out=out_dram[j, :, bh * nb : (bh + 1) * nb], in_=ot[:, :]
                    )
    nc.compile()
    return nc


def _get_nc(path, bl):
    key = (path, bl)
    if key not in _CACHE:
        _CACHE[key] = _build(path, bl)
    return _CACHE[key]


def _split_hilo(a):
    import ml_dtypes

    hi = a.astype(ml_dtypes.bfloat16)
    lo = (a - hi.astype(np.float32)).astype(ml_dtypes.bfloat16)
    return hi, lo


def _prep_core_inputs(x, W, b, path, n_cores, bl):
    """Shard batch across cores; relayout x slice to [JC, P, bl] d-major."""
    # W: [J, D, K] -> [P, JC*K] with wt[d, (j*C+c)*K + k] = W[j, c*128+d, k]
    wt = np.ascontiguousarray(
        W.reshape(J, C, P, K).transpose(2, 0, 1, 3).reshape(P, JC * K)
    )
    bias = np.ascontiguousarray(b.T)  # [K, J]

    if path == "hilo":
        wt_hi, wt_lo = _split_hilo(wt)

    in_maps = []
    for m in range(n_cores):
        xs = x[m * bl : (m + 1) * bl, 1 : J + 1, :]  # [bl, J, D] view
        # -> [J, D, bl] -> [JC, P, bl]
        xt = np.ascontiguousarray(xs.transpose(1, 2, 0)).reshape(JC, P, bl)
        if path == "hilo":
            xt_hi, xt_lo = _split_hilo(xt)
            in_maps.append(
                {
                    "xt_hi": xt_hi,
                    "xt_lo": xt_lo,
                    "wt_hi": wt_hi,
                    "wt_lo": wt_lo,
                    "bias": bias,
                }
            )
        else:
            in_maps.append({"xt": xt, "wt": wt, "bias": bias})
    return in_maps


def _gather(results, n_cores, bl):
    # per-core out [J, K, bl] -> full [B, J, K]
    out = np.empty((n_cores * bl, J, K), dtype=np.float32)
    for m, r in enumerate(results):
        out[m * bl : (m + 1) * bl] = r["out"].transpose(2, 0, 1)
    return out


def _ensure_ntff_hook():
    """The agent image's antenv lacks axon_hooks; shim it so trace=True can
    register the NTFF profiling hook (see trn_agent_boot.trn_boot)."""
    import sys
    import types

    try:
        from antenv.axon_hooks import get_axon_ntff_profile_hook  # noqa: F401

        return
    except ImportError:
        pass
    import antenv

    mod = types.ModuleType("antenv.axon_hooks")
    mod._hook = None

    def set_axon_ntff_profile_hook(h):
        mod._hook = h

    def get_axon_ntff_profile_hook():
        return mod._hook

    mod.set_axon_ntff_profile_hook = set_axon_ntff_profile_hook
    mod.get_axon_ntff_profile_hook = get_axon_ntff_profile_hook
    sys.modules["antenv.axon_hooks"] = mod
    antenv.axon_hooks = mod
    try:
        from trn_agent_boot.trn_boot import _ntff_profile_via_ctypes

        hook = _ntff_profile_via_ctypes("/opt/axon/libaxon_pjrt.so")
        if hook is not None:
            mod._hook = hook
    except Exception:
        pass


def run(x, W, b, path=None, trace=False, n_cores=M_CORES, bl=None):
    from concourse.bass_utils import run_bass_kernel_spmd

    if trace:
        _ensure_ntff_hook()

    path = path or _PATH
    bl = bl or (x.shape[0] // n_cores)
    x = np.asarray(x, dtype=np.float32)
    W = np.asarray(W, dtype=np.float32)
    b = np.asarray(b, dtype=np.float32)
    nc = _get_nc(path, bl)
    in_maps = _prep_core_inputs(x, W, b, path, n_cores, bl)
    res = run_bass_kernel_spmd(
        nc, in_maps, core_ids=list(range(n_cores)), trace=trace
    )
    out = _gather(res.results, n_cores, bl)
    return out, res


def kernel(x, W, b):
    out, _ = run(x, W, b)
    return out
